# revision 9
# baseline (speedup 1.0000x reference)
"""MoE routing kernel (nn_BrainModel_1640677507517) for 8x TRN2 NeuronCores.

Math (per token x[10]):
  h1 = relu(x @ W1[e] + b1[e])          e=0..7, 32 units   -> 256 feats
  h2 = relu(h1 @ W2[e] + b2[e])                            -> 256 feats
  outs = tanh(h2 @ W3[e] + b3[e])                          -> 8x2
  g = relu(x @ G1 + gb1); w = softmax(g @ G2 + gb2)        -> 8
  fused = sum_e outs[e]*w[e]                               -> 2
  y = tanh(relu([fused,x] @ R1 + rb1) @ R2 + rb2)          -> 2

v2 design: fp8e4m3 DoubleRow matmuls (0.5 cyc/row) for the expert MLP and
gating (2-level hi+lo fp8 weights packed in the DoubleRow slot pair, rhs
duplicated via a stride-0 broadcast AP); bf16 for the refine/selector tail.
Layer-1 biases ride as ones-rows appended to the x stream. Tokens stay on
the free dim (512/tile); two tiles of a pair are interleaved on partitions
(feature d of tile q at partition 2d+q) so one DMA loads a stacked pair.
Per-batch (4 tiles) tail: ew/outs packed in [128,NT] tiles, one bf16
selector matmul produces softmax numerators (rows 32:40) and denominators
(rows 64:72) plus the refine output psum (rows 0:8) in a single PSUM bank.
Pure data parallel over 8 cores; weights replicated.
"""

import numpy as np
import ml_dtypes

import concourse.bacc as bacc
import concourse.tile as tile
from concourse import mybir
from concourse.bass_utils import run_bass_kernel_spmd

N_CORES = 8
B = 1048576
BS = B // N_CORES  # 131072 tokens per core
NT = 512           # tokens per tile
NTILES = BS // NT  # 256
NB = NTILES // 4   # 64 batches of 4 tiles

F32 = mybir.dt.float32
BF = mybir.dt.bfloat16
F8 = mybir.dt.float8e4
DR = mybir.MatmulPerfMode.DoubleRow
AF = mybir.ActivationFunctionType
ADD, MAX = mybir.AluOpType.add, mybir.AluOpType.max
MULT = mybir.AluOpType.mult

E4 = ml_dtypes.float8_e4m3fn if hasattr(ml_dtypes, "float8_e4m3fn") \
    else ml_dtypes.float8_e4m3

# --- fp8 weight pack wc8 [128, C8]; DR block = hi at +0, lo at +128 ---
_o8 = 0


def _c8(n):
    global _o8
    o = _o8
    _o8 += n
    return o


C_W1 = [[_c8(256) for _h in range(2)] for _q in range(2)]  # [q][half] 22-row
C_G1 = _c8(256)     # pair block [22, 2, 128]
C_W2A = _c8(256)    # [128, 2, 128]
C_W2B = _c8(256)
C_W3J = [_c8(256) for _j in range(4)]  # [128, 2, 128] per tile, 1-level
C_G2 = [_c8(256) for _p in range(2)]  # [128, 2, 128] per pair
C8 = ((_o8 + 31) // 32) * 32

# --- bf16 weight pack wcb [128, CBW] ---
_ob = 0


def _cb(n):
    global _ob
    o = _ob
    _ob += n
    return o


CB_R1X = _cb(128)   # [22, 128] interleaved-row x-part of R1 (+rb1 ones-rows)
CB_R1F = [_cb(128) for _p in range(2)]  # [8, 128] at partitions 32:40
CB_R2 = [_cb(16) for _p in range(2)]    # [128, 8] per pair (pad 16)
CB_SEL = _cb(80)    # [128, 72] selector (pad 80)
CBW = ((_ob + 15) // 16) * 16

# --- f32 bias pack cbf [128, CF] ---
CF_B2A, CF_B2B, CF_B3, CF_GB2, CF_RB2 = range(5)
CF = 8

TRACE = False
LAST_RESULTS = None


def _q8(a):
    return np.asarray(a, dtype=np.float32).astype(E4).astype(np.float32)


def _pack_consts(W1, b1, W2, b2, W3, b3, G1, gb1, G2, gb2, R1, rb1, R2, rb2):
    wc8 = np.zeros((128, C8), dtype=np.float32)

    def put2(rows, cols, w):
        # write hi at cols, lo at cols+128 (2-level fp8)
        w = np.asarray(w, dtype=np.float32)
        hi = _q8(w)
        wc8[rows, cols] = hi
        wc8[rows, [c + 128 for c in cols]] = _q8(w - hi)

    # W1 blocks: tile q rows 2d+q (d=0..9), ones-row at 20+q carries b1
    w1f = np.transpose(W1, (1, 0, 2)).reshape(10, 256)  # [d, 256]
    for q in range(2):
        rows = [2 * d + q for d in range(10)]
        for h in range(2):
            c0 = C_W1[q][h]
            put2(np.repeat(rows, 128),
                 np.tile(np.arange(c0, c0 + 128), 10),
                 w1f[:, 128 * h:128 * h + 128].reshape(-1))
            put2([20 + q] * 128, list(range(c0, c0 + 128)),
                 b1.reshape(-1)[128 * h:128 * h + 128])
    # G1 pair block: tile q -> out cols 64q:64q+64
    for q in range(2):
        rows = [2 * d + q for d in range(10)]
        cs = np.arange(C_G1 + 64 * q, C_G1 + 64 * q + 64)
        put2(np.repeat(rows, 64), np.tile(cs, 10), G1.reshape(-1))
        put2([20 + q] * 64, list(cs), gb1)
    # W2 block-diag [128, 128] per half
    for h, c0 in ((0, C_W2A), (1, C_W2B)):
        blk = np.zeros((128, 128), dtype=np.float32)
        for e in range(4):
            blk[32 * e:32 * e + 32, 32 * e:32 * e + 32] = W2[4 * h + e]
        rr, cc = np.nonzero(np.ones_like(blk))
        put2(rr, cc + c0, blk.reshape(-1))
    # W3 single-level per tile j: zero-padded M=128 blocks accumulated into
    # psO4 (DoubleRow cannot write at a psum partition offset)
    for j in range(4):
        for s in range(2):
            for e in range(4):
                for a in range(2):
                    col = C_W3J[j] + 128 * s + 32 * j + 8 * s + 2 * e + a
                    wc8[32 * e:32 * e + 32, col] = _q8(W3[4 * s + e][:, a])
    # G2 pair blocks: rhs g2 rows 64q?? psg rows 0:64 tile 2p, 64:128 tile 2p+1
    for p in range(2):
        for q in range(2):
            j = 2 * p + q
            rows = np.arange(64 * q, 64 * q + 64)
            # dup cols aligned with outs rows + plain cols for denominator
            for e in range(8):
                for a in range(2):
                    col = C_G2[p] + 32 * j + (2 * e + a if e < 4
                                              else 8 + 2 * (e - 4) + a)
                    put2(rows, [col] * 64, G2[:, e])
            for e in range(8):
                col = C_G2[p] + 32 * j + 16 + e
                put2(rows, [col] * 64, G2[:, e])

    wcb = np.zeros((128, CBW), dtype=np.float32)
    # R1X: tile q rows 2d+q = R1[2+d], ones-row 20+q = rb1; out cols 64q+
    for q in range(2):
        for d in range(10):
            wcb[2 * d + q, CB_R1X + 64 * q:CB_R1X + 64 * q + 64] = R1[2 + d]
        wcb[20 + q, CB_R1X + 64 * q:CB_R1X + 64 * q + 64] = rb1
    # R1F blocks at partition rows 32:40 (rhs fused4[32:40])
    for p in range(2):
        for q in range(2):
            j = 2 * p + q
            for a in range(2):
                wcb[32 + 2 * j + a,
                    CB_R1F[p] + 64 * q:CB_R1F[p] + 64 * q + 64] = R1[a]
    # R2 blocks: col 2q+a <- rows 64q+(0:64) = R2[:, a]
    for p in range(2):
        for q in range(2):
            for a in range(2):
                wcb[64 * q:64 * q + 64,
                    CB_R2[p] + 2 * (2 * p + q) + a] = R2[:, a]
    # selector [128, 72]: col 32+2j+a nums; col 64+2j+a dens
    for j in range(4):
        for r in range(16):
            wcb[32 * j + r, CB_SEL + 32 + 2 * j + (r % 2)] = 1.0
        for a in range(2):
            wcb[32 * j + 16:32 * j + 24, CB_SEL + 64 + 2 * j + a] = 1.0

    cbf = np.zeros((128, CF), dtype=np.float32)
    cbf[:, CF_B2A] = b2[0:4].reshape(-1)
    cbf[:, CF_B2B] = b2[4:8].reshape(-1)
    for j in range(4):
        for e in range(8):
            for a in range(2):
                r = 32 * j + (2 * e + a if e < 4 else 8 + 2 * (e - 4) + a)
                cbf[r, CF_B3] = b3[e, a]
                cbf[r, CF_GB2] = gb2[e]
        cbf[32 * j + 16:32 * j + 24, CF_B3] = 20.0
        cbf[32 * j + 16:32 * j + 24, CF_GB2] = gb2
    for j in range(4):
        for a in range(2):
            cbf[2 * j + a, CF_RB2] = rb2[a]

    return (wc8.astype(E4), wcb.astype(ml_dtypes.bfloat16), cbf)


def _build_bass():
    nc = bacc.Bacc("TRN2", debug=False, enable_asserts=False,
                   num_devices=N_CORES)
    x8e = nc.dram_tensor("x8e", [11, BS], F8, kind="ExternalInput").ap()
    xbe = nc.dram_tensor("xbe", [11, BS], BF, kind="ExternalInput").ap()
    wc8 = nc.dram_tensor("wc8", [128, C8], F8, kind="ExternalInput").ap()
    wcb = nc.dram_tensor("wcb", [128, CBW], BF, kind="ExternalInput").ap()
    cbf = nc.dram_tensor("cbf", [128, CF], F32, kind="ExternalInput").ap()
    y_out = nc.dram_tensor("y_out", [NB, 8, NT], F32,
                           kind="ExternalOutput").ap()

    with tile.TileContext(nc) as tc:
        with (
            tc.tile_pool(name="const", bufs=1) as cp,
            tc.tile_pool(name="io", bufs=3) as iop,
            tc.tile_pool(name="act", bufs=2) as ap_,
            tc.tile_pool(name="psL", bufs=3, space="PSUM") as pL,
            tc.tile_pool(name="psO", bufs=1, space="PSUM") as pO,
            tc.tile_pool(name="psWT", bufs=1, space="PSUM") as pWT,
        ):
            C8t = cp.tile([128, C8], F8)
            nc.sync.dma_start(C8t[:, :], wc8[:, :])
            CBt = cp.tile([128, CBW], BF)
            nc.sync.dma_start(CBt[:, :], wcb[:, :])
            CFt = cp.tile([128, CF], F32)
            nc.sync.dma_start(CFt[:, :], cbf[:, :])

            # AP helpers for DoubleRow lhsT blocks (slot-major: hi | lo)
            def lhs8(c0, k, m, w=128):
                return C8t[0:k, c0:c0 + 2 * w].rearrange(
                    "p (two m) -> p two m", two=2)[:, :, 0:m]

            def bcast(apx, k):
                return apx.unsqueeze(1).broadcast_to([k, 2, NT])

            def head_pair(b, p, st):
                """pair p of batch b: expert MLP for 2 tiles + gating."""
                t0 = 4 * b + 2 * p
                x8s = iop.tile([22, NT], F8, tag="x8s")
                nc.sync.dma_start(
                    x8s[:, :],
                    x8e[:, t0 * NT:(t0 + 2) * NT].rearrange(
                        "p (i n) -> p i n", i=2))
                if p == 0:
                    # prefetch this batch's xbs pair tiles (used in tail_b)
                    st['xbs'] = []
                    for pp_ in range(2):
                        xbs = iop.tile([22, NT], BF, tag="xbs")
                        tb = 4 * b + 2 * pp_
                        nc.sync.dma_start(
                            xbs[:, :],
                            xbe[:, tb * NT:(tb + 2) * NT].rearrange(
                                "p (i n) -> p i n", i=2))
                        st['xbs'].append(xbs)

                # gating psum first in rotation so G1 can fill the W1->W2 gap
                psg = pL.tile([128, NT], F32, tag="L", name="psg")
                ps1s = []
                h1ts = []
                for q in range(2):
                    ps1 = pL.tile([128, 2, NT], F32, tag="L", name="ps1")
                    for h in range(2):
                        nc.tensor.matmul(ps1[:, h, :],
                                         lhs8(C_W1[q][h], 22, 128),
                                         bcast(x8s[:, :], 22),
                                         start=True, stop=True, perf_mode=DR)
                    h1t = ap_.tile([128, 2, NT], F8, tag="h1", bufs=3)
                    nc.scalar.activation(h1t[:, :, :], ps1[:, :, :], AF.Relu)
                    ps1s.append(ps1)
                    h1ts.append(h1t)
                nc.tensor.matmul(psg[:, :], lhs8(C_G1, 22, 128),
                                 bcast(x8s[:, :], 22),
                                 start=True, stop=True, perf_mode=DR)
                g2t = ap_.tile([128, NT], F8, tag="g2", bufs=2)
                nc.vector.tensor_scalar(g2t[:, :], psg[:, :], 0.0, 0.0,
                                        ADD, MAX)
                st['g2t'][p] = g2t

                for q in range(2):
                    j = 2 * p + q
                    ps2 = pL.tile([128, 2, NT], F32, tag="L", name="ps2")
                    nc.tensor.matmul(ps2[:, 0, :], lhs8(C_W2A, 128, 128),
                                     bcast(h1ts[q][:, 0, :], 128),
                                     start=True, stop=True, perf_mode=DR)
                    nc.tensor.matmul(ps2[:, 1, :], lhs8(C_W2B, 128, 128),
                                     bcast(h1ts[q][:, 1, :], 128),
                                     start=True, stop=True, perf_mode=DR)
                    h2t = ap_.tile([128, 2, NT], F8, tag="h2", bufs=3)
                    if q == 0:
                        nc.vector.tensor_scalar(h2t[:, 0, :], ps2[:, 0, :],
                                                CFt[:, CF_B2A:CF_B2A + 1],
                                                0.0, ADD, MAX)
                        nc.vector.tensor_scalar(h2t[:, 1, :], ps2[:, 1, :],
                                                CFt[:, CF_B2B:CF_B2B + 1],
                                                0.0, ADD, MAX)
                    else:
                        nc.scalar.activation(h2t[:, 0, :], ps2[:, 0, :],
                                             AF.Relu,
                                             bias=CFt[:, CF_B2A:CF_B2A + 1])
                        nc.vector.tensor_scalar(h2t[:, 1, :], ps2[:, 1, :],
                                                CFt[:, CF_B2B:CF_B2B + 1],
                                                0.0, ADD, MAX)
                    nc.tensor.matmul(st['psO4'][:, :],
                                     lhs8(C_W3J[j], 128, 128),
                                     h2t[:, :, :],
                                     start=(j == 0), stop=(j == 3),
                                     perf_mode=DR)

            def gating_mm(st):
                """both G2 passes at end of head1 (psW4 slot freed by then)."""
                st['psW4'] = pWT.tile([128, NT], F32, tag="wt", name="psW4")
                for p in range(2):
                    nc.tensor.matmul(st['psW4'][:, :],
                                     lhs8(C_G2[p], 128, 128),
                                     bcast(st['g2t'][p][:, :], 128),
                                     start=(p == 0), stop=(p == 1),
                                     perf_mode=DR)

            def tail_a(s):
                """outs/ew/ewp/selND/rcp/fused for batch state s."""
                outs4 = ap_.tile([128, NT], BF, tag="outs4")
                nc.scalar.activation(outs4[:, :], s['psO4'][:, :], AF.Tanh,
                                     bias=CFt[:, CF_B3:CF_B3 + 1])
                ew4 = ap_.tile([128, NT], BF, tag="ew4")
                nc.scalar.activation(ew4[:, :], s['psW4'][:, :], AF.Exp,
                                     bias=CFt[:, CF_GB2:CF_GB2 + 1])
                ewp = ap_.tile([128, NT], BF, tag="ewp")
                nc.gpsimd.tensor_tensor(ewp[:, :], ew4[:, :], outs4[:, :],
                                        MULT)
                s['tail'] = pWT.tile([128, NT], F32, tag="wt", name="tail")
                nc.tensor.matmul(s['tail'][0:72, :],
                                 CBt[:, CB_SEL:CB_SEL + 72],
                                 ewp[:, :], start=True, stop=True)
                rcp8 = ap_.tile([40, NT], F32, tag="rcp8")
                nc.vector.reciprocal(rcp8[32:40, :], s['tail'][64:72, :])
                fused4 = ap_.tile([40, NT], BF, tag="fused4")
                nc.vector.tensor_tensor(fused4[32:40, :], s['tail'][32:40, :],
                                        rcp8[32:40, :], MULT)
                s['fused4'] = fused4

            def tail_b(s):
                """refine chain + y store for batch state s."""
                b = s['b']
                fused4 = s['fused4']
                for p in range(2):
                    xbs = s['xbs'][p]
                    psr1 = pL.tile([128, NT], F32, tag="L", name="psr1")
                    nc.tensor.matmul(psr1[:, :],
                                     CBt[0:22, CB_R1X:CB_R1X + 128],
                                     xbs[:, :], start=True, stop=False)
                    nc.tensor.matmul(psr1[:, :],
                                     CBt[32:40, CB_R1F[p]:CB_R1F[p] + 128],
                                     fused4[32:40, :], start=False, stop=True)
                    r2t = ap_.tile([128, NT], BF, tag="r2")
                    if p == 0:
                        nc.scalar.activation(r2t[:, :], psr1[:, :], AF.Relu)
                    else:
                        nc.vector.tensor_scalar(r2t[:, :], psr1[:, :], 0.0,
                                                0.0, ADD, MAX)
                    nc.tensor.matmul(s['tail'][0:8, :],
                                     CBt[:, CB_R2[p]:CB_R2[p] + 8],
                                     r2t[:, :], start=(p == 0), stop=(p == 1))
                yt8 = ap_.tile([8, NT], F32, tag="yt8")
                nc.scalar.activation(yt8[:, :], s['tail'][0:8, :], AF.Tanh,
                                     bias=CFt[0:8, CF_RB2:CF_RB2 + 1])
                nc.sync.dma_start(y_out[b, :, :], yt8[:, :])

            prev = None
            for b in range(NB):
                st = {'b': b, 'g2t': [None, None],
                      'psO4': pO.tile([128, NT], F32, tag="o4", name="psO4")}
                if prev is not None:
                    tail_a(prev)
                head_pair(b, 0, st)
                if prev is not None:
                    tail_b(prev)
                head_pair(b, 1, st)
                gating_mm(st)
                prev = st
            tail_a(prev)
            tail_b(prev)
    nc.compile()
    return nc


_NC_CACHE = None


def kernel(x, W1, b1, W2, b2, W3, b3, G1, gb1, G2, gb2, R1, rb1, R2, rb2):
    global _NC_CACHE, LAST_RESULTS
    x = np.asarray(x, dtype=np.float32)
    wc8, wcb, cbf = _pack_consts(
        np.asarray(W1), np.asarray(b1), np.asarray(W2), np.asarray(b2),
        np.asarray(W3), np.asarray(b3), np.asarray(G1), np.asarray(gb1),
        np.asarray(G2), np.asarray(gb2), np.asarray(R1), np.asarray(rb1),
        np.asarray(R2), np.asarray(rb2))
    if _NC_CACHE is None:
        _NC_CACHE = _build_bass()
    nc = _NC_CACHE

    xt = np.ascontiguousarray(x.T)  # [10, B]
    ones = np.ones((1, B), dtype=np.float32)
    xe = np.concatenate([xt, ones], axis=0)  # [11, B]
    x8e_full = xe.astype(E4)
    xbe_full = xe.astype(ml_dtypes.bfloat16)

    in_maps = []
    for c in range(N_CORES):
        sl = slice(c * BS, (c + 1) * BS)
        in_maps.append({
            "x8e": np.ascontiguousarray(x8e_full[:, sl]),
            "xbe": np.ascontiguousarray(xbe_full[:, sl]),
            "wc8": wc8, "wcb": wcb, "cbf": cbf,
        })
    res = run_bass_kernel_spmd(nc, in_maps, core_ids=list(range(N_CORES)),
                               trace=TRACE)
    LAST_RESULTS = res
    # y_out [NB, 8, NT]: row 2j+a, col n -> token (4b+j)*NT+n, action a
    ys = []
    for c in range(N_CORES):
        yo = res.results[c]["y_out"]          # [NB, 8, NT]
        yo = yo.reshape(NB, 4, 2, NT)          # [b, j, a, n]
        yo = np.transpose(yo, (0, 1, 3, 2))    # [b, j, n, a]
        ys.append(yo.reshape(BS, 2))
    return np.concatenate(ys, axis=0).astype(np.float32)


# revision 19
# speedup vs baseline: 1.0285x; 1.0285x over previous
"""MoE routing kernel (nn_BrainModel_1640677507517) for 8x TRN2 NeuronCores.

Math (per token x[10]):
  h1 = relu(x @ W1[e] + b1[e])          e=0..7, 32 units   -> 256 feats
  h2 = relu(h1 @ W2[e] + b2[e])                            -> 256 feats
  outs = tanh(h2 @ W3[e] + b3[e])                          -> 8x2
  g = relu(x @ G1 + gb1); w = softmax(g @ G2 + gb2)        -> 8
  fused = sum_e outs[e]*w[e]                               -> 2
  y = tanh(relu([fused,x] @ R1 + rb1) @ R2 + rb2)          -> 2

Layout: feature-major (features on SBUF partitions, tokens on the free dim),
float32r matmul operands (1 cycle/row on the PE vs 4 for plain fp32).
The narrow softmax/refine tail is batched over 4 token-tiles at 32-aligned
partition blocks so elementwise ops run with ~full lane occupancy.
Host pre-transposes x -> x_t [10, B_shard]; output returns as y_t
[2, B_shard] and is transposed back on the host. Pure data parallel over
8 cores; weights replicated.
"""

import numpy as np

import concourse.bacc as bacc
import concourse.tile as tile
from concourse import mybir
from concourse.bass_utils import run_bass_kernel_spmd

N_CORES = 8
B = 1048576
BS = B // N_CORES  # 131072 tokens per core
NT = 512           # tokens per tile
NTILES = BS // NT  # 256 (multiple of 4)

F32 = mybir.dt.float32
FR = mybir.dt.float32r
AF = mybir.ActivationFunctionType

# --- weight-pack column layout (wc: [128, CC] float32r) ---
_off = 0


def _col(n):
    global _off
    o = _off
    _off += n
    return o


C_W1A = _col(128)   # [10,128]   W1 experts 0-3
C_W1B = _col(128)   # [10,128]   W1 experts 4-7
C_G1 = _col(64)     # [10,64]
C_W2A = _col(128)   # [128,128]  block-diag W2 experts 0-3
C_W2B = _col(128)   # [128,128]  block-diag W2 experts 4-7
C_G1X = [_col(128) for _ in range(2)]   # [10,128] G1 at cols 64q, rest 0
C_W3X = [_col(128) for _ in range(8)]   # [128,128] W3 A/B per tile j at
#                                         cols 32j (zero elsewhere)
C_G2DD = [_col(128) for _ in range(2)]  # [128,128] dup G2 block-diag, pair p
C_NUMS = _col(98)   # [128,98]   numerator selectors (4 tiles)
C_DENS = _col(98)   # [128,98]   denominator selectors (4 tiles)
C_R1X = [_col(128) for _ in range(2)]   # [10,128] R1 x-part at cols 64q
C_R1FA = _col(128)  # [98,128]   R1 rows 0:2 for pair0 (tiles 0,1 of batch)
C_R1FB = _col(128)  # [98,128]   R1 rows 0:2 for pair1 (tiles 2,3)
C_R2BD = [_col(36) for _ in range(2)]   # [128,36] R2 block-diag at cols 32p
CC = ((_off + 31) // 32) * 32

# --- bias pack (wb: [128, CB] float32) ---
C_B1A, C_B1B, C_B2A, C_B2B, C_GB1X2, C_B3O4, C_GB2D4, C_GB2P4, C_RB1X2, \
    C_RB24 = range(10)
CB = 16

TRACE = False
LAST_RESULTS = None


def _pack_consts(W1, b1, W2, b2, W3, b3, G1, gb1, G2, gb2, R1, rb1, R2, rb2):
    wc = np.zeros((128, CC), dtype=np.float32)
    w1 = np.transpose(W1, (1, 0, 2)).reshape(10, 256)
    wc[0:10, C_W1A:C_W1A + 128] = w1[:, 0:128]
    wc[0:10, C_W1B:C_W1B + 128] = w1[:, 128:256]
    wc[0:10, C_G1:C_G1 + 64] = G1
    for q in range(2):
        wc[0:10, C_G1X[q] + 64 * q:C_G1X[q] + 64 * q + 64] = G1
    for e in range(4):
        wc[e * 32:(e + 1) * 32, C_W2A + e * 32:C_W2A + (e + 1) * 32] = W2[e]
        wc[e * 32:(e + 1) * 32, C_W2B + e * 32:C_W2B + (e + 1) * 32] = W2[e + 4]
    # outs block row r (within a 32-row tile block): r = 2e+a (e<4),
    # r = 8+2(e-4)+a (e>=4); all other columns zero (the 8 W3 matmuls
    # accumulate into psO4 [128, NT], each contributing its own block)
    for j in range(4):
        for e in range(4):
            wc[e * 32:(e + 1) * 32,
               C_W3X[2 * j] + 32 * j + 2 * e:
               C_W3X[2 * j] + 32 * j + 2 * e + 2] = W3[e]
            wc[e * 32:(e + 1) * 32,
               C_W3X[2 * j + 1] + 32 * j + 8 + 2 * e:
               C_W3X[2 * j + 1] + 32 * j + 8 + 2 * e + 2] = W3[e + 4]
    # G2 pair block-diags over g2 [128, NT] (even tile rows 0:64, odd 64:128):
    # dup: out block col c<16 -> G2[:, c//2]; plain: col c<8 -> G2[:, c]
    for p in range(2):
        for c in range(16):
            wc[0:64, C_G2DD[p] + 64 * p + c] = G2[:, c // 2]
            wc[64:128, C_G2DD[p] + 64 * p + 32 + c] = G2[:, c // 2]
    # numerator selector: col 32j+a sums ewp rows 32j+r (r<16, r%2==a)
    for j in range(4):
        for r in range(16):
            wc[32 * j + r, C_NUMS + 32 * j + (r % 2)] = 1.0
    # denominator selector: every col of block j sums the 16 dup'd exp rows
    # of ew4 with coefficient 0.5 (each expert appears twice); identical
    # cols keep psD finite everywhere for the reciprocal
    for j in range(4):
        ncols = min(32, 98 - 32 * j)
        for c in range(ncols):
            wc[32 * j:32 * j + 16, C_DENS + 32 * j + c] = 0.5
    for q in range(2):
        wc[0:10, C_R1X[q] + 64 * q:C_R1X[q] + 64 * q + 64] = R1[2:12]
    # fused part of R1: fused4 rows 0:2/32:34/64:66/96:98 are tiles 0..3
    wc[0:2, C_R1FA:C_R1FA + 64] = R1[0:2]
    wc[32:34, C_R1FA + 64:C_R1FA + 128] = R1[0:2]
    wc[64:66, C_R1FB:C_R1FB + 64] = R1[0:2]
    wc[96:98, C_R1FB + 64:C_R1FB + 128] = R1[0:2]
    for p in range(2):
        wc[0:64, C_R2BD[p] + 32 * p:C_R2BD[p] + 32 * p + 2] = R2
        wc[64:128, C_R2BD[p] + 32 * p + 2:C_R2BD[p] + 32 * p + 4] = R2

    wb = np.zeros((128, CB), dtype=np.float32)
    wb[0:128, C_B1A] = b1[0:4].reshape(-1)
    wb[0:128, C_B1B] = b1[4:8].reshape(-1)
    wb[0:128, C_B2A] = b2[0:4].reshape(-1)
    wb[0:128, C_B2B] = b2[4:8].reshape(-1)
    wb[0:64, C_GB1X2] = gb1
    wb[64:128, C_GB1X2] = gb1
    b3f = b3.reshape(-1)
    for j in range(4):
        wb[32 * j:32 * j + 16, C_B3O4] = b3f
        wb[32 * j:32 * j + 16, C_GB2D4] = np.repeat(gb2, 2)
        wb[32 * j:32 * j + 8, C_GB2P4] = gb2
    wb[0:64, C_RB1X2] = rb1
    wb[64:128, C_RB1X2] = rb1
    for rr in (0, 2, 32, 34):
        wb[rr:rr + 2, C_RB24] = rb2
    return wc, wb


def _build_bass():
    nc = bacc.Bacc("TRN2", debug=False, enable_asserts=False, num_devices=N_CORES)
    x_t = nc.dram_tensor("x_t", [10, BS], FR, kind="ExternalInput").ap()
    wc = nc.dram_tensor("wc", [128, CC], FR, kind="ExternalInput").ap()
    wb = nc.dram_tensor("wb", [128, CB], F32, kind="ExternalInput").ap()
    y_t = nc.dram_tensor("y_t", [2, BS], F32, kind="ExternalOutput").ap()
    ADD, MAX = mybir.AluOpType.add, mybir.AluOpType.max

    with tile.TileContext(nc) as tc:
        with (
            tc.tile_pool(name="const", bufs=1) as cp,
            tc.tile_pool(name="io", bufs=12) as iop,
            tc.tile_pool(name="act", bufs=6) as ap_,
            tc.tile_pool(name="ps", bufs=1, space="PSUM") as pp,
        ):
            C = cp.tile([128, CC], FR)
            nc.sync.dma_start(C[:, :], wc[:, :])
            Cb = cp.tile([128, CB], F32)
            nc.sync.dma_start(Cb[:, :], wb[:, :])
            # PE warm-up consuming the const DMA so steady-state matmuls
            # carry at most one semaphore wait
            pwm = pp.tile([1, 8], F32, tag="bt", bufs=3)
            nc.tensor.matmul(pwm[:, :], C[0:1, 0:1], C[0:1, 0:8],
                             start=True, stop=True)

            def head(b, part_cb=None):
                """Per-4-tile-batch: input DMAs, expert MLP layers, gating,
                logits; returns carried state for tail(). part_cb(k) is
                invoked between pairs to interleave the previous batch's
                tail work."""
                xcs = []
                psO4 = pp.tile([128, NT], F32, tag="bt", bufs=3)
                psW4 = pp.tile([128, NT], F32, tag="bt", bufs=3)
                for p in range(2):  # pair index
                    psg = pp.tile([128, NT], F32, tag="psg", bufs=1)
                    for q in range(2):
                        j = 2 * p + q
                        if part_cb is not None:
                            part_cb(j)
                        i = 4 * b + j
                        sl = slice(i * NT, (i + 1) * NT)
                        xc = iop.tile([10, NT], FR, tag="xc")
                        nc.sync.dma_start(xc[0:10, :], x_t[:, sl])
                        xcs.append(xc)

                        ps1a = pp.tile([128, NT], F32, tag="psL1", bufs=2)
                        nc.tensor.matmul(ps1a[:, :], C[0:10, C_W1A:C_W1A + 128],
                                         xc[0:10, :], start=True, stop=True)
                        ps1b = pp.tile([128, NT], F32, tag="psL1", bufs=2)
                        nc.tensor.matmul(ps1b[:, :], C[0:10, C_W1B:C_W1B + 128],
                                         xc[0:10, :], start=True, stop=True)
                        nc.tensor.matmul(psg[:, :],
                                         C[0:10, C_G1X[q]:C_G1X[q] + 128],
                                         xc[0:10, :], start=(q == 0),
                                         stop=(q == 1))

                        h1a = ap_.tile([128, NT], FR, tag="h1a")
                        nc.scalar.activation(h1a[:, :], ps1a[:, :], AF.Relu,
                                             bias=Cb[0:128, C_B1A:C_B1A + 1])
                        h1b = ap_.tile([128, NT], FR, tag="h1b")
                        nc.vector.tensor_scalar(h1b[:, :], ps1b[:, :],
                                                Cb[0:128, C_B1B:C_B1B + 1], 0.0,
                                                ADD, MAX)

                        ps2a = pp.tile([128, NT], F32, tag="psL2", bufs=2)
                        nc.tensor.matmul(ps2a[:, :], C[:, C_W2A:C_W2A + 128],
                                         h1a[:, :], start=True, stop=True)
                        ps2b = pp.tile([128, NT], F32, tag="psL2", bufs=2)
                        nc.tensor.matmul(ps2b[:, :], C[:, C_W2B:C_W2B + 128],
                                         h1b[:, :], start=True, stop=True)
                        h2a = ap_.tile([128, NT], FR, tag="h2a")
                        nc.scalar.activation(h2a[:, :], ps2a[:, :], AF.Relu,
                                             bias=Cb[0:128, C_B2A:C_B2A + 1])
                        h2b = ap_.tile([128, NT], FR, tag="h2b")
                        nc.vector.tensor_scalar(h2b[:, :], ps2b[:, :],
                                                Cb[0:128, C_B2B:C_B2B + 1], 0.0,
                                                ADD, MAX)

                        # outs block j accumulates into psO4 (8 matmuls)
                        ca = C_W3X[2 * j]
                        cb = C_W3X[2 * j + 1]
                        nc.tensor.matmul(psO4[:, :], C[:, ca:ca + 128],
                                         h2a[:, :], start=(j == 0),
                                         stop=False)
                        nc.tensor.matmul(psO4[:, :], C[:, cb:cb + 128],
                                         h2b[:, :], start=False,
                                         stop=(j == 3))

                    # gating pair: relu then dup/plain logit matmuls
                    g2 = ap_.tile([128, NT], FR, tag="g2")
                    nc.scalar.activation(g2[:, :], psg[:, :], AF.Relu,
                                         bias=Cb[0:128, C_GB1X2:C_GB1X2 + 1])
                    cd = C_G2DD[p]
                    nc.tensor.matmul(psW4[:, :], C[:, cd:cd + 128], g2[:, :],
                                     start=(p == 0), stop=(p == 1))

                outs4 = ap_.tile([128, NT], FR, tag="outs4")
                nc.scalar.activation(outs4[:, :], psO4[:, :], AF.Tanh,
                                     bias=Cb[0:128, C_B3O4:C_B3O4 + 1])
                ew4 = ap_.tile([128, NT], FR, tag="ew4")
                nc.scalar.activation(ew4[:, :], psW4[:, :], AF.Exp,
                                     bias=Cb[0:128, C_GB2D4:C_GB2D4 + 1])
                return b, xcs, outs4, ew4

            def tail_a0(st):
                b, xcs, outs4, ew4 = st
                ewp4 = ap_.tile([128, NT], FR, tag="ewp4")
                nc.vector.tensor_mul(ewp4[:, :], ew4[:, :], outs4[:, :])
                psR4 = pp.tile([98, NT], F32, tag="bt", bufs=3)
                nc.tensor.matmul(psR4[:, :], C[:, C_NUMS:C_NUMS + 98],
                                 ewp4[:, :], start=True, stop=True)
                psD4 = pp.tile([98, NT], F32, tag="bt", bufs=3)
                nc.tensor.matmul(psD4[:, :], C[:, C_DENS:C_DENS + 98],
                                 ew4[:, :], start=True, stop=True)
                return psR4, psD4

            def tail_a1(ps):
                psR4, psD4 = ps
                rcp4 = ap_.tile([98, NT], F32, tag="rcp4")
                nc.vector.reciprocal(rcp4[:, :], psD4[:, :])
                fused4 = ap_.tile([98, NT], FR, tag="fused4")
                nc.vector.tensor_mul(fused4[:, :], psR4[:, :], rcp4[:, :])
                return fused4

            def tail_b(st, fused4, psY4, prange):
                b, xcs, outs4, ew4 = st
                for p in prange:
                    psr1 = pp.tile([128, NT], F32, tag="psg", bufs=1)
                    for q in range(2):
                        cr = C_R1X[q]
                        nc.tensor.matmul(psr1[:, :], C[0:10, cr:cr + 128],
                                         xcs[2 * p + q][0:10, :],
                                         start=(q == 0), stop=False)
                    c_r1f = C_R1FA if p == 0 else C_R1FB
                    nc.tensor.matmul(psr1[:, :], C[0:98, c_r1f:c_r1f + 128],
                                     fused4[:, :], start=False, stop=True)
                    r2 = ap_.tile([128, NT], FR, tag="r2")
                    nc.vector.tensor_scalar(r2[:, :], psr1[:, :],
                                            Cb[0:128, C_RB1X2:C_RB1X2 + 1],
                                            0.0, ADD, MAX)
                    c2 = C_R2BD[p]
                    nc.tensor.matmul(psY4[:, :], C[:, c2:c2 + 36], r2[:, :],
                                     start=(p == 0), stop=(p == 1))
            def tail_c(st, psY4):
                b = st[0]
                yt4 = ap_.tile([36, NT], F32, tag="yt4")
                nc.scalar.activation(yt4[:, :], psY4[:, :], AF.Tanh,
                                     bias=Cb[0:36, C_RB24:C_RB24 + 1])
                for j in range(4):
                    i = 4 * b + j
                    sl = slice(i * NT, (i + 1) * NT)
                    rr = (0, 2, 32, 34)[j]
                    nc.sync.dma_start(y_t[:, sl], yt4[rr:rr + 2, :])

            carried = None
            for b in range(NTILES // 4):
                state = {}

                def cb(j, _c=carried, _s=state):
                    if _c is None:
                        return
                    if j == 1:
                        _s['ps'] = tail_a0(_c)
                    elif j == 2:
                        _s['f'] = tail_a1(_s['ps'])
                        _s['y'] = pp.tile([36, NT], F32, tag="bt", bufs=3, name="psY4")
                    elif j == 3:
                        tail_b(_c, _s['f'], _s['y'], (0,))
                st = head(b, cb)
                if carried is not None:
                    tail_b(carried, state['f'], state['y'], (1,))
                    tail_c(carried, state['y'])
                carried = st
            ps = tail_a0(carried)
            f = tail_a1(ps)
            y4 = pp.tile([36, NT], F32, tag="bt", bufs=3, name="psY4e")
            tail_b(carried, f, y4, (0, 1))
            tail_c(carried, y4)
    nc.compile()
    return nc


_NC_CACHE = None


def kernel(x, W1, b1, W2, b2, W3, b3, G1, gb1, G2, gb2, R1, rb1, R2, rb2):
    global _NC_CACHE, LAST_RESULTS
    x = np.asarray(x)
    wc, wb = _pack_consts(np.asarray(W1), np.asarray(b1), np.asarray(W2),
                          np.asarray(b2), np.asarray(W3), np.asarray(b3),
                          np.asarray(G1), np.asarray(gb1), np.asarray(G2),
                          np.asarray(gb2), np.asarray(R1), np.asarray(rb1),
                          np.asarray(R2), np.asarray(rb2))
    if _NC_CACHE is None:
        _NC_CACHE = _build_bass()
    nc = _NC_CACHE
    in_maps = []
    for c in range(N_CORES):
        xs = np.ascontiguousarray(x[c * BS:(c + 1) * BS].T)
        in_maps.append({"x_t": xs, "wc": wc, "wb": wb})
    res = run_bass_kernel_spmd(nc, in_maps, core_ids=list(range(N_CORES)),
                               trace=TRACE)
    LAST_RESULTS = res
    y = np.concatenate([res.results[c]["y_t"].T for c in range(N_CORES)], axis=0)
    return y

